# revision 14
# baseline (speedup 1.0000x reference)
"""MLA (multi-headed latent attention) forward on 8 Trainium2 NeuronCores.

Sharding: data-parallel over batch (4) x tensor-parallel over heads (2):
core c handles batch c//2 with heads [16*(c%2), 16*(c%2)+16).
Each core computes a partial (H-dim) output contribution; host sums the
TP pair and stacks batches.

v2: all matmuls in bf16 (f32 PSUM accumulation), every weight DMA is a
single fully-contiguous block (host pre-arranges [p, k, c] layouts),
phase-1 contraction accumulates all 32 k-tiles directly in PSUM.
"""

import numpy as np
import ml_dtypes
import concourse.bass as bass
import concourse.mybir as mybir
import concourse.tile as tile
from concourse import bacc
from concourse import bass_utils

F32 = mybir.dt.float32
BF16 = mybir.dt.bfloat16
AX = mybir.AxisListType
OP = mybir.AluOpType
AF = mybir.ActivationFunctionType
NPBF = ml_dtypes.bfloat16

B, S, H, NH = 4, 1024, 4096, 32
QL, KVL, RD, ND, VD = 1536, 512, 64, 128, 128
QHD = ND + RD  # 192
EPS = 1e-6
NCORES = 8
TP = 2                 # tensor-parallel ways (heads)
HPC = NH // TP         # 16 heads per core
G = 2                  # heads per group
NG = HPC // G          # 8 groups
TOKT = S // 128        # 8 token tiles
KH = H // 128          # 32 contraction tiles for H
MT = 17                # wa m-tiles: 12 qa + 4 kv + 1 pe(64, zero-padded)
SCALE = float(QHD) ** -0.5

# rope feature permutation: pairs (d, d+32) land 16 lanes apart within a
# 32-partition quadrant so stream_shuffle can do rotate_half.
DIMS_PERM = np.array(
    list(range(0, 16)) + list(range(32, 48))
    + list(range(16, 32)) + list(range(48, 64)), dtype=np.int64)
SHUF_MASK = [(i + 16) % 32 for i in range(32)]

# pe first: its output also carries the folded LN means (wa cols 64/65 of
# the pe m-tile are wbar_qa/QL and wbar_kv/KVL), needed by both LN finalizes.
M_TILES = ([("pe", 0)] + [("qa", i) for i in range(12)]
           + [("kv", i) for i in range(4)])

_NC_CACHE = {}


def _build_nc():
    nc = bacc.Bacc("TRN2", target_bir_lowering=False, debug=False)

    def din(name, shape, dt=BF16):
        return nc.dram_tensor(name, shape, dt, kind="ExternalInput").ap()

    hs_r = din("hs_r", (128, KH * S))                # [p, k*t]
    wa_r = din("wa_r", (MT, 128, KH * 128))          # [m][p, k*c]
    wqb_r = din("wqb_r", (NG, 3, 128, 12 * 128))     # [g][m][p, k*c]
    wkvbk_r = din("wkvbk_r", (NG, 2, 128, 4 * 128))
    wkvbv_r = din("wkvbv_r", (NG, 128, 4 * 256))
    wo_r = din("wo_r", (KH, 128, 16 * 128))          # [hr][p, m*c]
    csq = din("csq", (128, S))
    ssq = din("ssq", (128, S))
    tri = din("tri", (128, 128))
    ones_in = din("ones_in", (128, 1))
    outT = nc.dram_tensor("outT", (H, S), F32, kind="ExternalOutput").ap()

    with tile.TileContext(nc) as tc:
        with tc.tile_pool(name="pers", bufs=1) as pers:
            # ---------------- persistent tiles ----------------
            q_anT = pers.tile([128, 12 * S], BF16)     # LN(q_a)^T  (1536, 1024)
            kv_cnT = pers.tile([128, 4 * S], BF16)     # LN(kv_c)^T (512, 1024)
            kpeT2 = pers.tile([128, S], BF16)          # roped k_pe^T, both halves
            csq_t = pers.tile([128, S], BF16)
            ssq_t = pers.tile([128, S], BF16)
            tri_t = pers.tile([128, 128], BF16)
            ones_t = pers.tile([128, 1], BF16)
            nc.sync.dma_start(out=csq_t[:, :], in_=csq)
            nc.sync.dma_start(out=ssq_t[:, :], in_=ssq)
            nc.sync.dma_start(out=tri_t[:, :], in_=tri)
            nc.sync.dma_start(out=ones_t[:, :], in_=ones_in)

            # ======== phase 1: X^T = Wa^T @ hs^T (full-K PSUM accum), LN ========
            # hs arrives as 4 quarter tiles; the first two m-tiles (pe, qa0)
            # pipeline their k-accumulation across quarters so the PE starts
            # after the first quarter lands instead of the full 8.4MB.
            with tc.tile_pool(name="hsp", bufs=1) as hsp, \
                 tc.tile_pool(name="p1wa", bufs=3) as p1wa, \
                 tc.tile_pool(name="sqp", bufs=2) as sqp, \
                 tc.tile_pool(name="rowp", bufs=1) as rowp, \
                 tc.tile_pool(name="bcp", bufs=2) as bcp, \
                 tc.tile_pool(name="p1ps", bufs=4, space="PSUM") as p1ps, \
                 tc.tile_pool(name="stps", bufs=4, space="PSUM") as stps:
                hsq = [hsp.tile([128, 8, S], BF16, tag=f"hsq{i}",
                                name=f"hsq{i}") for i in range(4)]
                wts = {}
                for idx in (0, 1):
                    kind, mi = M_TILES[idx]
                    wts[idx] = p1wa.tile([128, KH, 128], BF16, tag="wa",
                                         name=f"wa_{kind}_{mi}")
                nc.sync.dma_start(
                    out=hsq[0][:, :, :],
                    in_=hs_r[:, 0:8 * S].rearrange("p (k t) -> p k t", k=8))
                for idx in (0, 1):
                    nc.sync.dma_start(
                        out=wts[idx][:, :, :],
                        in_=wa_r[idx].rearrange("p (k c) -> p k c", k=KH))
                for i in range(1, 4):
                    nc.sync.dma_start(
                        out=hsq[i][:, :, :],
                        in_=hs_r[:, i * 8 * S:(i + 1) * 8 * S]
                            .rearrange("p (k t) -> p k t", k=8))

                def mtile_dest(kind, mi):
                    if kind == "qa":
                        return q_anT[:, mi * S:(mi + 1) * S], 128
                    if kind == "kv":
                        return kv_cnT[:, mi * S:(mi + 1) * S], 128
                    return kpeT2[0:64, :], 64

                stats = {}

                def emit_epilogue(kind, mi, pst):
                    """copies + stats for a finished (m-tile, [ps_qh0, ps_qh1])"""
                    destm, rows = mtile_dest(kind, mi)
                    for qh in range(2):
                        sl = slice(qh * 512, qh * 512 + 512)
                        nc.scalar.copy(destm[:, sl], pst[qh][:rows, :])
                    if kind == "pe":
                        return
                    last = 11 if kind == "qa" else 3
                    if mi == 0:
                        stats[kind] = [
                            stps.tile([1, 512], F32, tag="st",
                                      name=f"st_{kind}_{j}") for j in range(4)]
                    st = stats[kind]
                    sq = sqp.tile([128, S], BF16, tag="sq")
                    nc.scalar.activation(sq[:, :], destm, AF.Square)
                    for qh in range(2):
                        sl = slice(qh * 512, qh * 512 + 512)
                        nc.tensor.matmul(
                            st[qh][:, :], ones_t[:, :], destm[:, sl],
                            start=(mi == 0), stop=(mi == last))
                        nc.tensor.matmul(
                            st[2 + qh][:, :], ones_t[:, :], sq[:, sl],
                            start=(mi == 0), stop=(mi == last))

                # --- intro: m-tiles 0 (pe) and 1 (qa0), quarter-pipelined ---
                intro_ps = {}
                for idx in (0, 1):
                    kind, mi = M_TILES[idx]
                    _, rows = mtile_dest(kind, mi)
                    intro_ps[idx] = [p1ps.tile([128, 512], F32, tag="p1",
                                               name=f"p1_intro_{idx}_{qh}")
                                     for qh in range(2)]
                for quarter in range(4):
                    for idx in (0, 1):
                        kind, mi = M_TILES[idx]
                        _, rows = mtile_dest(kind, mi)
                        for qh in range(2):
                            for k8 in range(8):
                                k = quarter * 8 + k8
                                nc.tensor.matmul(
                                    intro_ps[idx][qh][:rows, :],
                                    wts[idx][:, k, :rows],
                                    hsq[quarter][:, k8, qh * 512: qh * 512 + 512],
                                    start=(k == 0), stop=(k == KH - 1))
                for idx in (0, 1):
                    kind, mi = M_TILES[idx]
                    emit_epilogue(kind, mi, intro_ps[idx])

                # ---- rope k_pe in place on kpeT2[0:64], then duplicate ----
                kp_sh = sqp.tile([64, S], BF16, tag="kpsh")
                nc.vector.stream_shuffle(kp_sh[:, :], kpeT2[0:64, :], SHUF_MASK)
                nc.vector.tensor_tensor(out=kp_sh[:, :], in0=kp_sh[:, :],
                                        in1=ssq_t[:64, :], op=OP.mult)
                nc.vector.tensor_tensor(out=kpeT2[0:64, :], in0=kpeT2[0:64, :],
                                        in1=csq_t[:64, :], op=OP.mult)
                nc.vector.tensor_tensor(out=kpeT2[0:64, :], in0=kpeT2[0:64, :],
                                        in1=kp_sh[:, :], op=OP.add)
                nc.sync.dma_start(out=kpeT2[64:128, :], in_=kpeT2[0:64, :])

                # --- main loop: m-tiles 2..16, full-K accumulation ---
                for idx in range(2, len(M_TILES)):
                    kind, mi = M_TILES[idx]
                    wt = p1wa.tile([128, KH, 128], BF16, tag="wa",
                                   name=f"wa_{kind}_{mi}")
                    nc.sync.dma_start(
                        out=wt[:, :, :],
                        in_=wa_r[idx].rearrange("p (k c) -> p k c", k=KH))
                    destm, rows = mtile_dest(kind, mi)
                    pst = []
                    for qh in range(2):
                        ps = p1ps.tile([128, 512], F32, tag="p1")
                        for k in range(KH):
                            nc.tensor.matmul(
                                ps[:rows, :], wt[:, k, :rows],
                                hsq[k // 8][:, k % 8, qh * 512: qh * 512 + 512],
                                start=(k == 0), stop=(k == KH - 1))
                        pst.append(ps)
                    emit_epilogue(kind, mi, pst)

                # ---- LN: finalize stats, broadcast, apply ----
                for kind, nmt, n_feat, destT in (("qa", 12, QL, q_anT),
                                                 ("kv", 4, KVL, kv_cnT)):
                    st = stats[kind]
                    rows4 = rowp.tile([1, 4 * S], F32, tag="rows",
                                      name=f"rows_{kind}")
                    mrow, vrow, srow, rrow = (
                        rows4[:, i * S:(i + 1) * S] for i in range(4))
                    for qh in range(2):
                        sl = slice(qh * 512, qh * 512 + 512)
                        nc.vector.tensor_scalar_mul(mrow[:, sl], st[qh][:, :],
                                                    1.0 / n_feat)
                        nc.vector.tensor_scalar_mul(vrow[:, sl], st[2 + qh][:, :],
                                                    1.0 / n_feat)
                    # var = E[x^2] - mean^2 + eps ; rstd = 1/sqrt(var)
                    nc.vector.tensor_tensor(out=srow[:, :], in0=mrow[:, :],
                                            in1=mrow[:, :], op=OP.mult)
                    nc.vector.tensor_tensor(out=vrow[:, :], in0=vrow[:, :],
                                            in1=srow[:, :], op=OP.subtract)
                    nc.vector.tensor_scalar_add(vrow[:, :], vrow[:, :], EPS)
                    nc.scalar.activation(srow[:, :], vrow[:, :], AF.Sqrt)
                    nc.vector.reciprocal(rrow[:, :], srow[:, :])
                    mb = bcp.tile([128, S], F32, tag="bc", name=f"mb_{kind}")
                    rb_ = bcp.tile([128, S], F32, tag="bc", name=f"rb_{kind}")
                    nc.gpsimd.partition_broadcast(mb[:, :], mrow[:, :])
                    nc.gpsimd.partition_broadcast(rb_[:, :], rrow[:, :])
                    for mi in range(nmt):
                        dsl = destT[:, mi * S:(mi + 1) * S]
                        nc.vector.tensor_tensor(out=dsl, in0=dsl, in1=mb[:, :],
                                                op=OP.subtract)
                        nc.vector.tensor_tensor(out=dsl, in0=dsl, in1=rb_[:, :],
                                                op=OP.mult)

            # ======== phase 2: per-group projections + attention ========
            with tc.tile_pool(name="otp", bufs=1) as otp:
              oT = otp.tile([128, HPC * S], BF16)      # normalized o^T (2048, 1024)
              with tc.tile_pool(name="gq2", bufs=3) as gqp, \
                 tc.tile_pool(name="gkn", bufs=2) as gknp, \
                 tc.tile_pool(name="gv", bufs=2) as gvp, \
                 tc.tile_pool(name="wq", bufs=2) as wqp, \
                 tc.tile_pool(name="wk", bufs=2) as wkp, \
                 tc.tile_pool(name="wv", bufs=2) as wvp, \
                 tc.tile_pool(name="rshp", bufs=2) as rshp, \
                 tc.tile_pool(name="pp", bufs=3) as ppool, \
                 tc.tile_pool(name="rsp", bufs=2) as rsp, \
                 tc.tile_pool(name="rbp", bufs=2) as rbp, \
                 tc.tile_pool(name="pjps", bufs=2, space="PSUM") as pjps, \
                 tc.tile_pool(name="sps", bufs=2, space="PSUM") as sps, \
                 tc.tile_pool(name="ops", bufs=2, space="PSUM") as ops, \
                 tc.tile_pool(name="smps", bufs=2, space="PSUM") as smps:
                for g in range(NG):
                    # ---- q^T for this group: 3 m-tiles (2x nope, 1x pe pair) ----
                    qT = gqp.tile([128, 3 * S], BF16, tag="qT")
                    for m in range(3):
                        wt = wqp.tile([128, 12, 128], BF16, tag="wqb",
                                      name=f"wqb_{g}_{m}")
                        nc.sync.dma_start(
                            out=wt[:, :, :],
                            in_=wqb_r[g, m].rearrange("p (k c) -> p k c", k=12))
                        for qh in range(2):
                            ps = pjps.tile([128, 512], F32, tag="pj")
                            for k in range(12):
                                nc.tensor.matmul(
                                    ps[:, :], wt[:, k, :],
                                    q_anT[:, k * S + qh * 512: k * S + qh * 512 + 512],
                                    start=(k == 0), stop=(k == 11))
                            nc.scalar.copy(
                                qT[:, m * S + qh * 512: m * S + qh * 512 + 512],
                                ps[:, :])
                    # rope the pe tile (m=2): rows 0:64 = head0 pe, 64:128 = head1 pe
                    pe = qT[:, 2 * S:3 * S]
                    rsh = rshp.tile([128, S], BF16, tag="rsh")
                    nc.vector.stream_shuffle(rsh[:, :], pe, SHUF_MASK)
                    nc.vector.tensor_tensor(out=rsh[:, :], in0=rsh[:, :],
                                            in1=ssq_t[:, :], op=OP.mult)
                    nc.vector.tensor_tensor(out=pe, in0=pe, in1=csq_t[:, :],
                                            op=OP.mult)
                    nc.vector.tensor_tensor(out=pe, in0=pe, in1=rsh[:, :],
                                            op=OP.add)

                    # ---- k_nope^T: 2 m-tiles ----
                    knT = gknp.tile([128, 2 * S], BF16, tag="knT")
                    for m in range(2):
                        wt = wkp.tile([128, 4, 128], BF16, tag="wk",
                                      name=f"wk_{g}_{m}")
                        nc.sync.dma_start(
                            out=wt[:, :, :],
                            in_=wkvbk_r[g, m].rearrange("p (k c) -> p k c", k=4))
                        for qh in range(2):
                            ps = pjps.tile([128, 512], F32, tag="pj")
                            for k in range(4):
                                nc.tensor.matmul(
                                    ps[:, :], wt[:, k, :],
                                    kv_cnT[:, k * S + qh * 512: k * S + qh * 512 + 512],
                                    start=(k == 0), stop=(k == 3))
                            nc.scalar.copy(
                                knT[:, m * S + qh * 512: m * S + qh * 512 + 512],
                                ps[:, :])

                    # ---- v token-major: (128 tok, 8 toktile x 256 cols) ----
                    v_sb = gvp.tile([128, TOKT * G * VD], BF16, tag="v")
                    wv_t = wvp.tile([128, 4, 256], BF16, tag="wv", name=f"wv_{g}")
                    nc.sync.dma_start(
                        out=wv_t[:, :, :],
                        in_=wkvbv_r[g].rearrange("p (k c) -> p k c", k=4))
                    for t in range(TOKT):
                        ps = pjps.tile([128, 512], F32, tag="pj")
                        for k in range(4):
                            nc.tensor.matmul(
                                ps[:, :256],
                                kv_cnT[:, k * S + t * 128: k * S + (t + 1) * 128],
                                wv_t[:, k, :], start=(k == 0), stop=(k == 3))
                        nc.scalar.copy(v_sb[:, t * 256:(t + 1) * 256],
                                       ps[:, :256])

                    # ---- attention per head ----
                    for hh in range(G):
                        hg = g * G + hh
                        po = [ops.tile([128, 512], F32, tag="po",
                                       name=f"po_{hg}_{qh}") for qh in range(2)]
                        psm = [smps.tile([1, 512], F32, tag="psm",
                                         name=f"psm_{hg}_{qh}") for qh in range(2)]
                        for qh in range(2):
                            last_ik = 4 * qh + 3
                            for ik in range(last_ik + 1):
                                qstart = 128 * ik
                                lo = max(qstart, 512 * qh)
                                hi = 512 * (qh + 1)
                                w = hi - lo
                                ps_s = sps.tile([128, 512], F32, tag="ps")
                                nc.tensor.matmul(
                                    ps_s[:, :w],
                                    knT[:, hh * S + ik * 128: hh * S + (ik + 1) * 128],
                                    qT[:, hh * S + lo: hh * S + hi],
                                    start=True, stop=False)
                                nc.tensor.matmul(
                                    ps_s[:, :w],
                                    kpeT2[hh * 64:(hh + 1) * 64, ik * 128:(ik + 1) * 128],
                                    qT[hh * 64:(hh + 1) * 64, 2 * S + lo: 2 * S + hi],
                                    start=False, stop=True)
                                p = ppool.tile([128, 512], BF16, tag="p")
                                nc.scalar.activation(p[:, :w], ps_s[:, :w],
                                                     AF.Exp, scale=SCALE)
                                if lo == qstart:
                                    nc.vector.tensor_tensor(
                                        out=p[:, 0:128], in0=p[:, 0:128],
                                        in1=tri_t[:, :], op=OP.mult)
                                nc.tensor.matmul(
                                    psm[qh][:, lo - 512 * qh: hi - 512 * qh],
                                    ones_t[:, :], p[:, :w],
                                    start=(ik == 0), stop=(ik == last_ik))
                                nc.tensor.matmul(
                                    po[qh][:, lo - 512 * qh: hi - 512 * qh],
                                    v_sb[:, ik * 256 + hh * 128: ik * 256 + (hh + 1) * 128],
                                    p[:, :w],
                                    start=(ik == 0), stop=(ik == last_ik))
                        rs = rsp.tile([1, S], F32, tag="rs")
                        nc.vector.reciprocal(rs[:, 0:512], psm[0][:, :])
                        nc.vector.reciprocal(rs[:, 512:1024], psm[1][:, :])
                        rb = rbp.tile([128, S], F32, tag="rb")
                        nc.gpsimd.partition_broadcast(rb[:, :], rs[:, :])
                        for qh in range(2):
                            nc.vector.tensor_tensor(
                                out=oT[:, hg * S + qh * 512: hg * S + qh * 512 + 512],
                                in0=po[qh][:, :],
                                in1=rb[:, qh * 512: qh * 512 + 512], op=OP.mult)

            # ======== phase 3: out^T = Wo^T @ o ========
            with tc.tile_pool(name="wop", bufs=3) as wop, \
                 tc.tile_pool(name="op", bufs=3) as outp, \
                 tc.tile_pool(name="wops", bufs=2, space="PSUM") as wops:
                for hr in range(H // 128):
                    wt = wop.tile([128, 16, 128], BF16, tag="wo", name=f"wo_{hr}")
                    nc.sync.dma_start(
                        out=wt[:, :, :],
                        in_=wo_r[hr].rearrange("p (m c) -> p m c", m=16))
                    for qh in range(2):
                        ps = wops.tile([128, 512], F32, tag="pw")
                        for m in range(HPC * VD // 128):
                            nc.tensor.matmul(
                                ps[:, :], wt[:, m, :],
                                oT[:, m * S + qh * 512: m * S + qh * 512 + 512],
                                start=(m == 0), stop=(m == HPC * VD // 128 - 1))
                        ot = outp.tile([128, 512], F32, tag="out")
                        nc.scalar.copy(ot[:, :], ps[:, :])
                        nc.sync.dma_start(
                            out=outT[hr * 128:(hr + 1) * 128, qh * 512:(qh + 1) * 512],
                            in_=ot[:, :])
    nc.compile()
    return nc


def _to_pkc(w, n_k):
    """(n_k*128, C) -> contiguous (128, n_k*C) bf16 ([p, k*c] layout)."""
    kk, c = w.shape[0] // 128, w.shape[1]
    assert kk == n_k
    return np.ascontiguousarray(
        w.reshape(n_k, 128, c).transpose(1, 0, 2).reshape(128, n_k * c)
    ).astype(NPBF)


def _host_prep(inputs):
    hs = np.asarray(inputs["hidden_states"], np.float32)
    cos = np.asarray(inputs["cos"], np.float32)
    sin = np.asarray(inputs["sin"], np.float32)
    pid = np.asarray(inputs["position_ids"]).astype(np.int64)
    Wqa = np.asarray(inputs["Wqa"], np.float32)
    gqa = np.asarray(inputs["gqa"], np.float32)
    Wqb = np.asarray(inputs["Wqb"], np.float32)
    Wkva = np.asarray(inputs["Wkva"], np.float32)
    gkva = np.asarray(inputs["gkva"], np.float32)
    Wkvb = np.asarray(inputs["Wkvb"], np.float32)
    Wo = np.asarray(inputs["Wo"], np.float32)

    # Wa = [Wqa | Wkva(kv) | Wkva(pe, rope-permuted) | LN-mean cols | pad]
    # cols 2112/2113 (locals 64/65 of the pe m-tile) carry wbar_qa/QL and
    # wbar_kv/KVL so the pe projection also produces both LN means.
    pad = np.zeros((H, 64), np.float32)
    pad[:, 0] = Wqa.mean(axis=1)
    pad[:, 1] = Wkva[:, :KVL].mean(axis=1)
    wa = np.concatenate(
        [Wqa, Wkva[:, :KVL], Wkva[:, KVL:][:, DIMS_PERM], pad], axis=1)
    # fold LN gains into the B-projections (bias terms are zero per spec)
    Wqb = Wqb * gqa[:, None]
    Wkvb = Wkvb * gkva[:, None]

    # wa_r: (17, 128, 32*128) bf16, [m][p, k*c], ordered as M_TILES (pe first)
    wa_bf = wa.astype(NPBF)
    wa_r = np.ascontiguousarray(
        wa_bf.reshape(KH, 128, MT, 128).transpose(2, 1, 0, 3)
        .reshape(MT, 128, KH * 128)[[16] + list(range(16))])

    # sign pattern for the shuffle-based rotate_half
    sign = np.where(DIMS_PERM < RD // 2, -1.0, 1.0).astype(np.float32)[:, None]

    tri = np.zeros((128, 128), np.float32)
    kp, q = np.mgrid[0:128, 0:128]
    tri[q >= kp] = 1.0
    tri = tri.astype(NPBF)

    w4 = Wqb.reshape(QL, NH, QHD)
    wk4 = Wkvb.reshape(KVL, NH, ND + VD)

    tp_data = []
    for t in range(TP):
        heads = slice(t * HPC, (t + 1) * HPC)
        # Wqb: group-blocked [h0 nope | h1 nope | h0 pe' | h1 pe'] per group
        wq = w4[:, heads]                       # (QL, 16, 192)
        nope = wq[:, :, :ND]                    # (QL, 16, 128)
        pe = wq[:, :, ND:][:, :, DIMS_PERM]     # (QL, 16, 64) permuted
        blocks = []
        for g in range(NG):
            blocks.extend([nope[:, 2 * g], nope[:, 2 * g + 1],
                           pe[:, 2 * g], pe[:, 2 * g + 1]])
        wqb_c = np.concatenate(blocks, axis=1)  # (QL, 16*192=3072)
        # (12k,128p, 8g, 3m, 128c) -> (g, m, p, k*c)
        wqb_r = np.ascontiguousarray(
            wqb_c.astype(NPBF).reshape(12, 128, NG, 3, 128)
            .transpose(2, 3, 1, 0, 4).reshape(NG, 3, 128, 12 * 128))

        wkc = wk4[:, heads]
        wkvbk_c = wkc[:, :, :ND].reshape(KVL, HPC * ND)
        wkvbv_c = wkc[:, :, ND:].reshape(KVL, HPC * VD)
        wkvbk_r = np.ascontiguousarray(
            wkvbk_c.astype(NPBF).reshape(4, 128, NG, 2, 128)
            .transpose(2, 3, 1, 0, 4).reshape(NG, 2, 128, 4 * 128))
        wkvbv_r = np.ascontiguousarray(
            wkvbv_c.astype(NPBF).reshape(4, 128, NG, 256)
            .transpose(2, 1, 0, 3).reshape(NG, 128, 4 * 256))

        wo_c = Wo[t * HPC * VD:(t + 1) * HPC * VD]    # (2048, 4096)
        wo_r = np.ascontiguousarray(
            wo_c.astype(NPBF).reshape(16, 128, KH, 128)
            .transpose(2, 1, 0, 3).reshape(KH, 128, 16 * 128))
        tp_data.append((wqb_r, wkvbk_r, wkvbv_r, wo_r))

    per_core = []
    for c in range(NCORES):
        b, t = divmod(c, TP)
        wqb_r, wkvbk_r, wkvbv_r, wo_r = tp_data[t]

        cos_g = cos[pid[b]]                     # (S, RD)
        sin_g = sin[pid[b]]
        cosT = np.ascontiguousarray(cos_g.T[DIMS_PERM])   # (64, S)
        sinT = np.ascontiguousarray(sin_g.T[DIMS_PERM])
        csq_c = np.ascontiguousarray(np.vstack([cosT, cosT])).astype(NPBF)
        ssq_c = np.ascontiguousarray(np.vstack([sinT * sign, sinT * sign])).astype(NPBF)

        hsT = hs[b].T                            # (H, S)
        hs_rc = np.ascontiguousarray(
            hsT.astype(NPBF).reshape(KH, 128, S).transpose(1, 0, 2)
            .reshape(128, KH * S))

        per_core.append({
            "hs_r": hs_rc,
            "wa_r": wa_r,
            "wqb_r": wqb_r,
            "wkvbk_r": wkvbk_r,
            "wkvbv_r": wkvbv_r,
            "wo_r": wo_r,
            "csq": csq_c,
            "ssq": ssq_c,
            "tri": tri,
            "ones_in": np.ones((128, 1), NPBF),
        })
    return per_core


def kernel(**inputs):
    if "nc" not in _NC_CACHE:
        _NC_CACHE["nc"] = _build_nc()
    nc = _NC_CACHE["nc"]
    in_maps = _host_prep(inputs)
    res = bass_utils.run_bass_kernel_spmd(nc, in_maps, core_ids=list(range(NCORES)))
    outs = []
    for b in range(B):
        acc = res.results[TP * b]["outT"].astype(np.float32)
        for t in range(1, TP):
            acc = acc + res.results[TP * b + t]["outT"]
        outs.append(acc.T)
    return np.stack(outs, axis=0)


# revision 17
# speedup vs baseline: 1.1102x; 1.1102x over previous
"""MLA (multi-headed latent attention) forward on 8 Trainium2 NeuronCores.

Sharding: data-parallel over batch (4) x tensor-parallel over heads (2):
core c handles batch c//2 with heads [16*(c%2), 16*(c%2)+16).
Each core computes a partial (H-dim) output contribution; host sums the
TP pair and stacks batches.

v2: all matmuls in bf16 (f32 PSUM accumulation), every weight DMA is a
single fully-contiguous block (host pre-arranges [p, k, c] layouts),
phase-1 contraction accumulates all 32 k-tiles directly in PSUM.
"""

import numpy as np
import ml_dtypes
import concourse.bass as bass
import concourse.mybir as mybir
import concourse.tile as tile
from concourse import bacc
from concourse import bass_utils

F32 = mybir.dt.float32
BF16 = mybir.dt.bfloat16
AX = mybir.AxisListType
OP = mybir.AluOpType
AF = mybir.ActivationFunctionType
NPBF = ml_dtypes.bfloat16

B, S, H, NH = 4, 1024, 4096, 32
QL, KVL, RD, ND, VD = 1536, 512, 64, 128, 128
QHD = ND + RD  # 192
EPS = 1e-6
NCORES = 8
TP = 2                 # tensor-parallel ways (heads)
HPC = NH // TP         # 16 heads per core
G = 2                  # heads per group
NG = HPC // G          # 8 groups
TOKT = S // 128        # 8 token tiles
KH = H // 128          # 32 contraction tiles for H
MT = 17                # wa m-tiles: 12 qa + 4 kv + 1 pe(64, zero-padded)
SCALE = float(QHD) ** -0.5

# rope feature permutation: pairs (d, d+32) land 16 lanes apart within a
# 32-partition quadrant so stream_shuffle can do rotate_half.
DIMS_PERM = np.array(
    list(range(0, 16)) + list(range(32, 48))
    + list(range(16, 32)) + list(range(48, 64)), dtype=np.int64)
SHUF_MASK = [(i + 16) % 32 for i in range(32)]

# pe first: its output also carries the folded LN means (wa cols 64/65 of
# the pe m-tile are wbar_qa/QL and wbar_kv/KVL), needed by both LN finalizes.
M_TILES = ([("pe", 0)] + [("qa", i) for i in range(12)]
           + [("kv", i) for i in range(4)])

_NC_CACHE = {}


def _build_nc():
    nc = bacc.Bacc("TRN2", target_bir_lowering=False, debug=False)

    def din(name, shape, dt=BF16):
        return nc.dram_tensor(name, shape, dt, kind="ExternalInput").ap()

    hs_r = din("hs_r", (128, KH * S))                # [p, k*t]
    wa_r = din("wa_r", (MT, 128, KH * 128))          # [m][p, k*c]
    wqb_r = din("wqb_r", (NG, 3, 128, 12 * 128))     # [g][m][p, k*c]
    wkvbk_r = din("wkvbk_r", (NG, 2, 128, 4 * 128))
    wkvbv_r = din("wkvbv_r", (NG, 128, 4 * 256))
    wo_r = din("wo_r", (KH, 128, 16 * 128))          # [hr][p, m*c]
    csq = din("csq", (128, S))
    ssq = din("ssq", (128, S))
    tri = din("tri", (128, 128))
    ones_in = din("ones_in", (128, 1))
    outT = nc.dram_tensor("outT", (H, S), F32, kind="ExternalOutput").ap()

    with tile.TileContext(nc) as tc:
        with tc.tile_pool(name="pers", bufs=1) as pers:
            # ---------------- persistent tiles ----------------
            q_anT = pers.tile([128, 12 * S], BF16)     # LN(q_a)^T  (1536, 1024)
            kv_cnT = pers.tile([128, 4 * S], BF16)     # LN(kv_c)^T (512, 1024)
            kpeT2 = pers.tile([128, S], BF16)          # roped k_pe^T, both halves
            csq_t = pers.tile([128, S], BF16)
            ssq_t = pers.tile([128, S], BF16)
            tri_t = pers.tile([128, 128], BF16)
            ones_t = pers.tile([128, 1], BF16)
            nc.sync.dma_start(out=csq_t[:, :], in_=csq)
            nc.sync.dma_start(out=ssq_t[:, :], in_=ssq)
            nc.sync.dma_start(out=tri_t[:, :], in_=tri)
            nc.sync.dma_start(out=ones_t[:, :], in_=ones_in)

            # ======== phase 1: X^T = Wa^T @ hs^T (full-K PSUM accum), LN ========
            # hs arrives as 4 quarter tiles; the first two m-tiles (pe, qa0)
            # pipeline their k-accumulation across quarters so the PE starts
            # after the first quarter lands instead of the full 8.4MB.
            with tc.tile_pool(name="hsp", bufs=1) as hsp, \
                 tc.tile_pool(name="p1wa", bufs=3) as p1wa, \
                 tc.tile_pool(name="sqp", bufs=2) as sqp, \
                 tc.tile_pool(name="rowp", bufs=1) as rowp, \
                 tc.tile_pool(name="bcp", bufs=2) as bcp, \
                 tc.tile_pool(name="p1ps", bufs=4, space="PSUM") as p1ps, \
                 tc.tile_pool(name="stps", bufs=4, space="PSUM") as stps:
                hsq = [hsp.tile([128, 8, S], BF16, tag=f"hsq{i}",
                                name=f"hsq{i}") for i in range(4)]
                wts = {}
                for idx in (0, 1):
                    kind, mi = M_TILES[idx]
                    wts[idx] = p1wa.tile([128, KH, 128], BF16, tag="wa",
                                         name=f"wa_{kind}_{mi}")
                nc.sync.dma_start(
                    out=hsq[0][:, :, :],
                    in_=hs_r[:, 0:8 * S].rearrange("p (k t) -> p k t", k=8))
                for idx in (0, 1):
                    nc.sync.dma_start(
                        out=wts[idx][:, :, :],
                        in_=wa_r[idx].rearrange("p (k c) -> p k c", k=KH))
                for i in range(1, 4):
                    nc.sync.dma_start(
                        out=hsq[i][:, :, :],
                        in_=hs_r[:, i * 8 * S:(i + 1) * 8 * S]
                            .rearrange("p (k t) -> p k t", k=8))

                def mtile_dest(kind, mi):
                    if kind == "qa":
                        return q_anT[:, mi * S:(mi + 1) * S], 128
                    if kind == "kv":
                        return kv_cnT[:, mi * S:(mi + 1) * S], 128
                    return kpeT2[0:64, :], 64

                stats = {}

                def emit_epilogue(kind, mi, pst):
                    """copies + stats for a finished (m-tile, [ps_qh0, ps_qh1])"""
                    destm, rows = mtile_dest(kind, mi)
                    for qh in range(2):
                        sl = slice(qh * 512, qh * 512 + 512)
                        nc.scalar.copy(destm[:, sl], pst[qh][:rows, :])
                    if kind == "pe":
                        return
                    last = 11 if kind == "qa" else 3
                    if mi == 0:
                        stats[kind] = [
                            stps.tile([1, 512], F32, tag="st",
                                      name=f"st_{kind}_{j}") for j in range(4)]
                    st = stats[kind]
                    sq = sqp.tile([128, S], BF16, tag="sq")
                    nc.scalar.activation(sq[:, :], destm, AF.Square)
                    for qh in range(2):
                        sl = slice(qh * 512, qh * 512 + 512)
                        nc.tensor.matmul(
                            st[qh][:, :], ones_t[:, :], destm[:, sl],
                            start=(mi == 0), stop=(mi == last))
                        nc.tensor.matmul(
                            st[2 + qh][:, :], ones_t[:, :], sq[:, sl],
                            start=(mi == 0), stop=(mi == last))

                # --- intro: m-tiles 0 (pe) and 1 (qa0), quarter-pipelined ---
                intro_ps = {}
                for idx in (0, 1):
                    kind, mi = M_TILES[idx]
                    _, rows = mtile_dest(kind, mi)
                    intro_ps[idx] = [p1ps.tile([128, 512], F32, tag="p1",
                                               name=f"p1_intro_{idx}_{qh}")
                                     for qh in range(2)]
                for quarter in range(4):
                    for idx in (0, 1):
                        kind, mi = M_TILES[idx]
                        _, rows = mtile_dest(kind, mi)
                        for qh in range(2):
                            for k8 in range(8):
                                k = quarter * 8 + k8
                                nc.tensor.matmul(
                                    intro_ps[idx][qh][:rows, :],
                                    wts[idx][:, k, :rows],
                                    hsq[quarter][:, k8, qh * 512: qh * 512 + 512],
                                    start=(k == 0), stop=(k == KH - 1))
                for idx in (0, 1):
                    kind, mi = M_TILES[idx]
                    emit_epilogue(kind, mi, intro_ps[idx])

                # ---- rope k_pe in place on kpeT2[0:64], then duplicate ----
                kp_sh = sqp.tile([64, S], BF16, tag="kpsh")
                nc.vector.stream_shuffle(kp_sh[:, :], kpeT2[0:64, :], SHUF_MASK)
                nc.vector.tensor_tensor(out=kp_sh[:, :], in0=kp_sh[:, :],
                                        in1=ssq_t[:64, :], op=OP.mult)
                nc.vector.tensor_tensor(out=kpeT2[0:64, :], in0=kpeT2[0:64, :],
                                        in1=csq_t[:64, :], op=OP.mult)
                nc.vector.tensor_tensor(out=kpeT2[0:64, :], in0=kpeT2[0:64, :],
                                        in1=kp_sh[:, :], op=OP.add)
                nc.sync.dma_start(out=kpeT2[64:128, :], in_=kpeT2[0:64, :])

                # --- main loop: m-tiles 2..16, full-K accumulation ---
                for idx in range(2, len(M_TILES)):
                    kind, mi = M_TILES[idx]
                    wt = p1wa.tile([128, KH, 128], BF16, tag="wa",
                                   name=f"wa_{kind}_{mi}")
                    nc.sync.dma_start(
                        out=wt[:, :, :],
                        in_=wa_r[idx].rearrange("p (k c) -> p k c", k=KH))
                    destm, rows = mtile_dest(kind, mi)
                    pst = []
                    for qh in range(2):
                        ps = p1ps.tile([128, 512], F32, tag="p1")
                        for k in range(KH):
                            nc.tensor.matmul(
                                ps[:rows, :], wt[:, k, :rows],
                                hsq[k // 8][:, k % 8, qh * 512: qh * 512 + 512],
                                start=(k == 0), stop=(k == KH - 1))
                        pst.append(ps)
                    emit_epilogue(kind, mi, pst)

                # ---- LN: finalize stats, broadcast, apply ----
                for kind, nmt, n_feat, destT in (("qa", 12, QL, q_anT),
                                                 ("kv", 4, KVL, kv_cnT)):
                    st = stats[kind]
                    rows4 = rowp.tile([1, 4 * S], F32, tag="rows",
                                      name=f"rows_{kind}")
                    mrow, vrow, srow, rrow = (
                        rows4[:, i * S:(i + 1) * S] for i in range(4))
                    for qh in range(2):
                        sl = slice(qh * 512, qh * 512 + 512)
                        nc.vector.tensor_scalar_mul(mrow[:, sl], st[qh][:, :],
                                                    1.0 / n_feat)
                        nc.vector.tensor_scalar_mul(vrow[:, sl], st[2 + qh][:, :],
                                                    1.0 / n_feat)
                    # var = E[x^2] - mean^2 + eps ; rstd = 1/sqrt(var)
                    nc.vector.tensor_tensor(out=srow[:, :], in0=mrow[:, :],
                                            in1=mrow[:, :], op=OP.mult)
                    nc.vector.tensor_tensor(out=vrow[:, :], in0=vrow[:, :],
                                            in1=srow[:, :], op=OP.subtract)
                    nc.vector.tensor_scalar_add(vrow[:, :], vrow[:, :], EPS)
                    nc.scalar.activation(srow[:, :], vrow[:, :], AF.Sqrt)
                    nc.vector.reciprocal(rrow[:, :], srow[:, :])
                    mb = bcp.tile([128, S], F32, tag="bc", name=f"mb_{kind}")
                    rb_ = bcp.tile([128, S], F32, tag="bc", name=f"rb_{kind}")
                    nc.gpsimd.partition_broadcast(mb[:, :], mrow[:, :])
                    nc.gpsimd.partition_broadcast(rb_[:, :], rrow[:, :])
                    for mi in range(nmt):
                        dsl = destT[:, mi * S:(mi + 1) * S]
                        nc.vector.tensor_tensor(out=dsl, in0=dsl, in1=mb[:, :],
                                                op=OP.subtract)
                        nc.vector.tensor_tensor(out=dsl, in0=dsl, in1=rb_[:, :],
                                                op=OP.mult)

            # ======== phase 2: per-group projections + attention ========
            with tc.tile_pool(name="otp", bufs=1) as otp:
              oT = otp.tile([128, HPC * S], BF16)      # normalized o^T (2048, 1024)
              with tc.tile_pool(name="gq2", bufs=3) as gqp, \
                 tc.tile_pool(name="gkn", bufs=2) as gknp, \
                 tc.tile_pool(name="gv", bufs=2) as gvp, \
                 tc.tile_pool(name="wq", bufs=2) as wqp, \
                 tc.tile_pool(name="wk", bufs=2) as wkp, \
                 tc.tile_pool(name="wv", bufs=2) as wvp, \
                 tc.tile_pool(name="rshp", bufs=2) as rshp, \
                 tc.tile_pool(name="pp", bufs=3) as ppool, \
                 tc.tile_pool(name="rsp", bufs=2) as rsp, \
                 tc.tile_pool(name="rbp", bufs=2) as rbp, \
                 tc.tile_pool(name="pjps", bufs=2, space="PSUM") as pjps, \
                 tc.tile_pool(name="sps", bufs=2, space="PSUM") as sps, \
                 tc.tile_pool(name="ops", bufs=2, space="PSUM") as ops, \
                 tc.tile_pool(name="smps", bufs=2, space="PSUM") as smps:
                for g in range(NG):
                    # ---- q^T for this group: 3 m-tiles (2x nope, 1x pe pair) ----
                    qT = gqp.tile([128, 3 * S], BF16, tag="qT")
                    for m in range(3):
                        wt = wqp.tile([128, 12, 128], BF16, tag="wqb",
                                      name=f"wqb_{g}_{m}")
                        nc.sync.dma_start(
                            out=wt[:, :, :],
                            in_=wqb_r[g, m].rearrange("p (k c) -> p k c", k=12))
                        for qh in range(2):
                            ps = pjps.tile([128, 512], F32, tag="pj")
                            for k in range(12):
                                nc.tensor.matmul(
                                    ps[:, :], wt[:, k, :],
                                    q_anT[:, k * S + qh * 512: k * S + qh * 512 + 512],
                                    start=(k == 0), stop=(k == 11))
                            nc.scalar.copy(
                                qT[:, m * S + qh * 512: m * S + qh * 512 + 512],
                                ps[:, :])
                    # rope the pe tile (m=2): rows 0:64 = head0 pe, 64:128 = head1 pe
                    pe = qT[:, 2 * S:3 * S]
                    rsh = rshp.tile([128, S], BF16, tag="rsh")
                    nc.vector.stream_shuffle(rsh[:, :], pe, SHUF_MASK)
                    nc.vector.tensor_tensor(out=rsh[:, :], in0=rsh[:, :],
                                            in1=ssq_t[:, :], op=OP.mult)
                    nc.vector.tensor_tensor(out=pe, in0=pe, in1=csq_t[:, :],
                                            op=OP.mult)
                    nc.vector.tensor_tensor(out=pe, in0=pe, in1=rsh[:, :],
                                            op=OP.add)

                    # ---- k_nope^T: 2 m-tiles ----
                    knT = gknp.tile([128, 2 * S], BF16, tag="knT")
                    for m in range(2):
                        wt = wkp.tile([128, 4, 128], BF16, tag="wk",
                                      name=f"wk_{g}_{m}")
                        nc.sync.dma_start(
                            out=wt[:, :, :],
                            in_=wkvbk_r[g, m].rearrange("p (k c) -> p k c", k=4))
                        for qh in range(2):
                            ps = pjps.tile([128, 512], F32, tag="pj")
                            for k in range(4):
                                nc.tensor.matmul(
                                    ps[:, :], wt[:, k, :],
                                    kv_cnT[:, k * S + qh * 512: k * S + qh * 512 + 512],
                                    start=(k == 0), stop=(k == 3))
                            nc.scalar.copy(
                                knT[:, m * S + qh * 512: m * S + qh * 512 + 512],
                                ps[:, :])

                    # ---- v token-major: (128 tok, 8 toktile x 256 cols) ----
                    v_sb = gvp.tile([128, TOKT * G * VD], BF16, tag="v")
                    wv_t = wvp.tile([128, 4, 256], BF16, tag="wv", name=f"wv_{g}")
                    nc.sync.dma_start(
                        out=wv_t[:, :, :],
                        in_=wkvbv_r[g].rearrange("p (k c) -> p k c", k=4))
                    for t in range(TOKT):
                        ps = pjps.tile([128, 512], F32, tag="pj")
                        for k in range(4):
                            nc.tensor.matmul(
                                ps[:, :256],
                                kv_cnT[:, k * S + t * 128: k * S + (t + 1) * 128],
                                wv_t[:, k, :], start=(k == 0), stop=(k == 3))
                        nc.scalar.copy(v_sb[:, t * 256:(t + 1) * 256],
                                       ps[:, :256])

                    # ---- attention per head ----
                    for hh in range(G):
                        hg = g * G + hh
                        po = [ops.tile([128, 512], F32, tag="po",
                                       name=f"po_{hg}_{qh}") for qh in range(2)]
                        psm = [smps.tile([1, 512], F32, tag="psm",
                                         name=f"psm_{hg}_{qh}") for qh in range(2)]
                        for qh in range(2):
                            last_ik = 4 * qh + 3
                            for ik in range(last_ik + 1):
                                qstart = 128 * ik
                                lo = max(qstart, 512 * qh)
                                hi = 512 * (qh + 1)
                                w = hi - lo
                                ps_s = sps.tile([128, 512], F32, tag="ps")
                                nc.tensor.matmul(
                                    ps_s[:, :w],
                                    knT[:, hh * S + ik * 128: hh * S + (ik + 1) * 128],
                                    qT[:, hh * S + lo: hh * S + hi],
                                    start=True, stop=False)
                                nc.tensor.matmul(
                                    ps_s[:, :w],
                                    kpeT2[hh * 64:(hh + 1) * 64, ik * 128:(ik + 1) * 128],
                                    qT[hh * 64:(hh + 1) * 64, 2 * S + lo: 2 * S + hi],
                                    start=False, stop=True)
                                p = ppool.tile([128, 512], BF16, tag="p")
                                nc.scalar.activation(p[:, :w], ps_s[:, :w],
                                                     AF.Exp, scale=SCALE)
                                if lo == qstart:
                                    nc.vector.tensor_tensor(
                                        out=p[:, 0:128], in0=p[:, 0:128],
                                        in1=tri_t[:, :], op=OP.mult)
                                nc.tensor.matmul(
                                    psm[qh][:, lo - 512 * qh: hi - 512 * qh],
                                    ones_t[:, :], p[:, :w],
                                    start=(ik == 0), stop=(ik == last_ik))
                                nc.tensor.matmul(
                                    po[qh][:, lo - 512 * qh: hi - 512 * qh],
                                    v_sb[:, ik * 256 + hh * 128: ik * 256 + (hh + 1) * 128],
                                    p[:, :w],
                                    start=(ik == 0), stop=(ik == last_ik))
                        rs = rsp.tile([1, S], F32, tag="rs")
                        nc.vector.reciprocal(rs[:, 0:512], psm[0][:, :])
                        nc.vector.reciprocal(rs[:, 512:1024], psm[1][:, :])
                        rb = rbp.tile([128, S], F32, tag="rb")
                        nc.gpsimd.partition_broadcast(rb[:, :], rs[:, :])
                        for qh in range(2):
                            nc.vector.tensor_tensor(
                                out=oT[:, hg * S + qh * 512: hg * S + qh * 512 + 512],
                                in0=po[qh][:, :],
                                in1=rb[:, qh * 512: qh * 512 + 512], op=OP.mult)

            # ======== phase 3: out^T = Wo^T @ o ========
            with tc.tile_pool(name="wop", bufs=3) as wop, \
                 tc.tile_pool(name="op", bufs=3) as outp, \
                 tc.tile_pool(name="wops", bufs=2, space="PSUM") as wops:
                for hr in range(H // 128):
                    wt = wop.tile([128, 16, 128], BF16, tag="wo", name=f"wo_{hr}")
                    nc.sync.dma_start(
                        out=wt[:, :, :],
                        in_=wo_r[hr].rearrange("p (m c) -> p m c", m=16))
                    for qh in range(2):
                        ps = wops.tile([128, 512], F32, tag="pw")
                        for m in range(HPC * VD // 128):
                            nc.tensor.matmul(
                                ps[:, :], wt[:, m, :],
                                oT[:, m * S + qh * 512: m * S + qh * 512 + 512],
                                start=(m == 0), stop=(m == HPC * VD // 128 - 1))
                        ot = outp.tile([128, 512], F32, tag="out")
                        nc.scalar.copy(ot[:, :], ps[:, :])
                        nc.sync.dma_start(
                            out=outT[hr * 128:(hr + 1) * 128, qh * 512:(qh + 1) * 512],
                            in_=ot[:, :])
    nc.compile()
    return nc


def _to_pkc(w, n_k):
    """(n_k*128, C) -> contiguous (128, n_k*C) bf16 ([p, k*c] layout)."""
    kk, c = w.shape[0] // 128, w.shape[1]
    assert kk == n_k
    return np.ascontiguousarray(
        w.reshape(n_k, 128, c).transpose(1, 0, 2).reshape(128, n_k * c)
    ).astype(NPBF)


def _host_prep(inputs):
    hs = np.asarray(inputs["hidden_states"], np.float32)
    cos = np.asarray(inputs["cos"], np.float32)
    sin = np.asarray(inputs["sin"], np.float32)
    pid = np.asarray(inputs["position_ids"]).astype(np.int64)
    Wqa = np.asarray(inputs["Wqa"], np.float32)
    gqa = np.asarray(inputs["gqa"], np.float32)
    Wqb = np.asarray(inputs["Wqb"], np.float32)
    Wkva = np.asarray(inputs["Wkva"], np.float32)
    gkva = np.asarray(inputs["gkva"], np.float32)
    Wkvb = np.asarray(inputs["Wkvb"], np.float32)
    Wo = np.asarray(inputs["Wo"], np.float32)

    # Wa = [Wqa | Wkva(kv) | Wkva(pe, rope-permuted) | LN-mean cols | pad]
    # cols 2112/2113 (locals 64/65 of the pe m-tile) carry wbar_qa/QL and
    # wbar_kv/KVL so the pe projection also produces both LN means.
    pad = np.zeros((H, 64), np.float32)
    pad[:, 0] = Wqa.mean(axis=1)
    pad[:, 1] = Wkva[:, :KVL].mean(axis=1)
    wa = np.concatenate(
        [Wqa, Wkva[:, :KVL], Wkva[:, KVL:][:, DIMS_PERM], pad], axis=1)
    # fold LN gains into the B-projections (bias terms are zero per spec)
    Wqb = Wqb * gqa[:, None]
    Wkvb = Wkvb * gkva[:, None]

    # wa_r: (17, 128, 32*128) bf16, [m][p, k*c], ordered as M_TILES (pe first)
    wa_bf = wa.astype(NPBF)
    wa_r = np.ascontiguousarray(
        wa_bf.reshape(KH, 128, MT, 128).transpose(2, 1, 0, 3)
        .reshape(MT, 128, KH * 128)[[16] + list(range(16))])

    # sign pattern for the shuffle-based rotate_half
    sign = np.where(DIMS_PERM < RD // 2, -1.0, 1.0).astype(np.float32)[:, None]

    tri = np.zeros((128, 128), np.float32)
    kp, q = np.mgrid[0:128, 0:128]
    tri[q >= kp] = 1.0
    tri = tri.astype(NPBF)

    w4 = Wqb.reshape(QL, NH, QHD)
    wk4 = Wkvb.reshape(KVL, NH, ND + VD)

    tp_data = []
    for t in range(TP):
        heads = slice(t * HPC, (t + 1) * HPC)
        # Wqb: group-blocked [h0 nope | h1 nope | h0 pe' | h1 pe'] per group
        wq = w4[:, heads]                       # (QL, 16, 192)
        nope = wq[:, :, :ND]                    # (QL, 16, 128)
        pe = wq[:, :, ND:][:, :, DIMS_PERM]     # (QL, 16, 64) permuted
        blocks = []
        for g in range(NG):
            blocks.extend([nope[:, 2 * g], nope[:, 2 * g + 1],
                           pe[:, 2 * g], pe[:, 2 * g + 1]])
        wqb_c = np.concatenate(blocks, axis=1)  # (QL, 16*192=3072)
        # (12k,128p, 8g, 3m, 128c) -> (g, m, p, k*c)
        wqb_r = np.ascontiguousarray(
            wqb_c.astype(NPBF).reshape(12, 128, NG, 3, 128)
            .transpose(2, 3, 1, 0, 4).reshape(NG, 3, 128, 12 * 128))

        wkc = wk4[:, heads]
        wkvbk_c = wkc[:, :, :ND].reshape(KVL, HPC * ND)
        wkvbv_c = wkc[:, :, ND:].reshape(KVL, HPC * VD)
        wkvbk_r = np.ascontiguousarray(
            wkvbk_c.astype(NPBF).reshape(4, 128, NG, 2, 128)
            .transpose(2, 3, 1, 0, 4).reshape(NG, 2, 128, 4 * 128))
        wkvbv_r = np.ascontiguousarray(
            wkvbv_c.astype(NPBF).reshape(4, 128, NG, 256)
            .transpose(2, 1, 0, 3).reshape(NG, 128, 4 * 256))

        wo_c = Wo[t * HPC * VD:(t + 1) * HPC * VD]    # (2048, 4096)
        wo_r = np.ascontiguousarray(
            wo_c.astype(NPBF).reshape(16, 128, KH, 128)
            .transpose(2, 1, 0, 3).reshape(KH, 128, 16 * 128))
        tp_data.append((wqb_r, wkvbk_r, wkvbv_r, wo_r))

    per_core = []
    for c in range(NCORES):
        b, t = divmod(c, TP)
        wqb_r, wkvbk_r, wkvbv_r, wo_r = tp_data[t]

        cos_g = cos[pid[b]]                     # (S, RD)
        sin_g = sin[pid[b]]
        cosT = np.ascontiguousarray(cos_g.T[DIMS_PERM])   # (64, S)
        sinT = np.ascontiguousarray(sin_g.T[DIMS_PERM])
        csq_c = np.ascontiguousarray(np.vstack([cosT, cosT])).astype(NPBF)
        ssq_c = np.ascontiguousarray(np.vstack([sinT * sign, sinT * sign])).astype(NPBF)

        hsT = hs[b].T                            # (H, S)
        hs_rc = np.ascontiguousarray(
            hsT.astype(NPBF).reshape(KH, 128, S).transpose(1, 0, 2)
            .reshape(128, KH * S))

        per_core.append({
            "hs_r": hs_rc,
            "wa_r": wa_r,
            "wqb_r": wqb_r,
            "wkvbk_r": wkvbk_r,
            "wkvbv_r": wkvbv_r,
            "wo_r": wo_r,
            "csq": csq_c,
            "ssq": ssq_c,
            "tri": tri,
            "ones_in": np.ones((128, 1), NPBF),
        })
    return per_core


def kernel(**inputs):
    if "nc" not in _NC_CACHE:
        _NC_CACHE["nc"] = _build_nc()
    nc = _NC_CACHE["nc"]
    in_maps = _host_prep(inputs)
    res = bass_utils.run_bass_kernel_spmd(nc, in_maps, core_ids=list(range(NCORES)))
    outs = []
    for b in range(B):
        acc = res.results[TP * b]["outT"].astype(np.float32)
        for t in range(1, TP):
            acc = acc + res.results[TP * b + t]["outT"]
        outs.append(acc.T)
    return np.stack(outs, axis=0)


# revision 18
# speedup vs baseline: 1.1293x; 1.0172x over previous
"""MLA (multi-headed latent attention) forward on 8 Trainium2 NeuronCores.

Sharding: data-parallel over batch (4) x tensor-parallel over heads (2):
core c handles batch c//2 with heads [16*(c%2), 16*(c%2)+16).
Each core computes a partial (H-dim) output contribution; host sums the
TP pair and stacks batches.

v2: all matmuls in bf16 (f32 PSUM accumulation), every weight DMA is a
single fully-contiguous block (host pre-arranges [p, k, c] layouts),
phase-1 contraction accumulates all 32 k-tiles directly in PSUM.
"""

import numpy as np
import ml_dtypes
import concourse.bass as bass
import concourse.mybir as mybir
import concourse.tile as tile
from concourse import bacc
from concourse import bass_utils

F32 = mybir.dt.float32
BF16 = mybir.dt.bfloat16
AX = mybir.AxisListType
OP = mybir.AluOpType
AF = mybir.ActivationFunctionType
NPBF = ml_dtypes.bfloat16

B, S, H, NH = 4, 1024, 4096, 32
QL, KVL, RD, ND, VD = 1536, 512, 64, 128, 128
QHD = ND + RD  # 192
EPS = 1e-6
NCORES = 8
TP = 2                 # tensor-parallel ways (heads)
HPC = NH // TP         # 16 heads per core
G = 2                  # heads per group
NG = HPC // G          # 8 groups
TOKT = S // 128        # 8 token tiles
KH = H // 128          # 32 contraction tiles for H
MT = 17                # wa m-tiles: 12 qa + 4 kv + 1 pe(64, zero-padded)
SCALE = float(QHD) ** -0.5

# rope feature permutation: pairs (d, d+32) land 16 lanes apart within a
# 32-partition quadrant so stream_shuffle can do rotate_half.
DIMS_PERM = np.array(
    list(range(0, 16)) + list(range(32, 48))
    + list(range(16, 32)) + list(range(48, 64)), dtype=np.int64)
SHUF_MASK = [(i + 16) % 32 for i in range(32)]

# pe first: its output also carries the folded LN means (wa cols 64/65 of
# the pe m-tile are wbar_qa/QL and wbar_kv/KVL), needed by both LN finalizes.
M_TILES = ([("pe", 0)] + [("qa", i) for i in range(12)]
           + [("kv", i) for i in range(4)])

_NC_CACHE = {}


def _build_nc():
    nc = bacc.Bacc("TRN2", target_bir_lowering=False, debug=False)

    def din(name, shape, dt=BF16):
        return nc.dram_tensor(name, shape, dt, kind="ExternalInput").ap()

    hs_r = din("hs_r", (128, KH * S))                # [p, k*t]
    wa_r = din("wa_r", (MT, 128, KH * 128))          # [m][p, k*c]
    wqb_r = din("wqb_r", (NG, 3, 128, 12 * 128))     # [g][m][p, k*c]
    wkvbk_r = din("wkvbk_r", (NG, 2, 128, 4 * 128))
    wkvbv_r = din("wkvbv_r", (NG, 128, 4 * 256))
    wo_r = din("wo_r", (KH, 128, 16 * 128))          # [hr][p, m*c]
    csq = din("csq", (128, S))
    ssq = din("ssq", (128, S))
    tri = din("tri", (128, 128))
    ones_in = din("ones_in", (128, 1))
    outT = nc.dram_tensor("outT", (H, S), F32, kind="ExternalOutput").ap()

    with tile.TileContext(nc) as tc:
        with tc.tile_pool(name="pers", bufs=1) as pers:
            # ---------------- persistent tiles ----------------
            q_anT = pers.tile([128, 12 * S], BF16)     # LN(q_a)^T  (1536, 1024)
            kv_cnT = pers.tile([128, 4 * S], BF16)     # LN(kv_c)^T (512, 1024)
            kpeT2 = pers.tile([128, S], BF16)          # roped k_pe^T, both halves
            csq_t = pers.tile([128, S], BF16)
            ssq_t = pers.tile([128, S], BF16)
            tri_t = pers.tile([128, 128], BF16)
            ones_t = pers.tile([128, 1], BF16)

            # ======== phase 1: X^T = Wa^T @ hs^T (full-K PSUM accum), LN ========
            # hs arrives as 4 quarter tiles; the first two m-tiles (pe, qa0)
            # pipeline their k-accumulation across quarters so the PE starts
            # after the first quarter lands instead of the full 8.4MB.
            with tc.tile_pool(name="hsp", bufs=1) as hsp, \
                 tc.tile_pool(name="p1wa", bufs=3) as p1wa, \
                 tc.tile_pool(name="sqp", bufs=2) as sqp, \
                 tc.tile_pool(name="rowp", bufs=1) as rowp, \
                 tc.tile_pool(name="bcp", bufs=2) as bcp, \
                 tc.tile_pool(name="p1ps", bufs=4, space="PSUM") as p1ps, \
                 tc.tile_pool(name="stps", bufs=4, space="PSUM") as stps:
                hst = [hsp.tile([128, S], BF16, tag=f"hst{k}",
                                name=f"hst{k}") for k in range(KH)]
                wts = {}
                for idx in (0, 1):
                    kind, mi = M_TILES[idx]
                    wts[idx] = p1wa.tile([128, KH, 128], BF16, tag="wa",
                                         name=f"wa_{kind}_{mi}")
                for idx in (0, 1):
                    nc.sync.dma_start(
                        out=wts[idx][:, :, :],
                        in_=wa_r[idx].rearrange("p (k c) -> p k c", k=KH))
                for k in range(KH):
                    nc.sync.dma_start(out=hst[k][:, :],
                                      in_=hs_r[:, k * S:(k + 1) * S])
                nc.sync.dma_start(out=ones_t[:, :], in_=ones_in)
                nc.sync.dma_start(out=csq_t[:, :], in_=csq)
                nc.sync.dma_start(out=ssq_t[:, :], in_=ssq)
                nc.sync.dma_start(out=tri_t[:, :], in_=tri)

                def mtile_dest(kind, mi):
                    if kind == "qa":
                        return q_anT[:, mi * S:(mi + 1) * S], 128
                    if kind == "kv":
                        return kv_cnT[:, mi * S:(mi + 1) * S], 128
                    return kpeT2[0:64, :], 64

                stats = {}

                def emit_epilogue(kind, mi, pst):
                    """copies + stats for a finished (m-tile, [ps_qh0, ps_qh1])"""
                    destm, rows = mtile_dest(kind, mi)
                    for qh in range(2):
                        sl = slice(qh * 512, qh * 512 + 512)
                        nc.scalar.copy(destm[:, sl], pst[qh][:rows, :])
                    if kind == "pe":
                        return
                    last = 11 if kind == "qa" else 3
                    if mi == 0:
                        stats[kind] = [
                            stps.tile([1, 512], F32, tag="st",
                                      name=f"st_{kind}_{j}") for j in range(4)]
                    st = stats[kind]
                    sq = sqp.tile([128, S], BF16, tag="sq")
                    nc.scalar.activation(sq[:, :], destm, AF.Square)
                    for qh in range(2):
                        sl = slice(qh * 512, qh * 512 + 512)
                        nc.tensor.matmul(
                            st[qh][:, :], ones_t[:, :], destm[:, sl],
                            start=(mi == 0), stop=(mi == last))
                        nc.tensor.matmul(
                            st[2 + qh][:, :], ones_t[:, :], sq[:, sl],
                            start=(mi == 0), stop=(mi == last))

                # --- intro: m-tiles 0 (pe) and 1 (qa0), quarter-pipelined ---
                intro_ps = {}
                for idx in (0, 1):
                    kind, mi = M_TILES[idx]
                    _, rows = mtile_dest(kind, mi)
                    intro_ps[idx] = [p1ps.tile([128, 512], F32, tag="p1",
                                               name=f"p1_intro_{idx}_{qh}")
                                     for qh in range(2)]
                for k in range(KH):
                    for idx in (0, 1):
                        kind, mi = M_TILES[idx]
                        _, rows = mtile_dest(kind, mi)
                        for qh in range(2):
                            nc.tensor.matmul(
                                intro_ps[idx][qh][:rows, :],
                                wts[idx][:, k, :rows],
                                hst[k][:, qh * 512: qh * 512 + 512],
                                start=(k == 0), stop=(k == KH - 1))
                for idx in (0, 1):
                    kind, mi = M_TILES[idx]
                    emit_epilogue(kind, mi, intro_ps[idx])

                # ---- rope k_pe in place on kpeT2[0:64], then duplicate ----
                kp_sh = sqp.tile([64, S], BF16, tag="kpsh")
                nc.vector.stream_shuffle(kp_sh[:, :], kpeT2[0:64, :], SHUF_MASK)
                nc.vector.tensor_tensor(out=kp_sh[:, :], in0=kp_sh[:, :],
                                        in1=ssq_t[:64, :], op=OP.mult)
                nc.vector.tensor_tensor(out=kpeT2[0:64, :], in0=kpeT2[0:64, :],
                                        in1=csq_t[:64, :], op=OP.mult)
                nc.vector.tensor_tensor(out=kpeT2[0:64, :], in0=kpeT2[0:64, :],
                                        in1=kp_sh[:, :], op=OP.add)
                nc.sync.dma_start(out=kpeT2[64:128, :], in_=kpeT2[0:64, :])

                # --- main loop: m-tiles 2..16, full-K accumulation ---
                for idx in range(2, len(M_TILES)):
                    kind, mi = M_TILES[idx]
                    wt = p1wa.tile([128, KH, 128], BF16, tag="wa",
                                   name=f"wa_{kind}_{mi}")
                    nc.sync.dma_start(
                        out=wt[:, :, :],
                        in_=wa_r[idx].rearrange("p (k c) -> p k c", k=KH))
                    destm, rows = mtile_dest(kind, mi)
                    pst = []
                    for qh in range(2):
                        ps = p1ps.tile([128, 512], F32, tag="p1")
                        for k in range(KH):
                            nc.tensor.matmul(
                                ps[:rows, :], wt[:, k, :rows],
                                hst[k][:, qh * 512: qh * 512 + 512],
                                start=(k == 0), stop=(k == KH - 1))
                        pst.append(ps)
                    emit_epilogue(kind, mi, pst)

                # ---- LN: finalize stats, broadcast, apply ----
                for kind, nmt, n_feat, destT in (("qa", 12, QL, q_anT),
                                                 ("kv", 4, KVL, kv_cnT)):
                    st = stats[kind]
                    rows4 = rowp.tile([1, 4 * S], F32, tag="rows",
                                      name=f"rows_{kind}")
                    mrow, vrow, srow, rrow = (
                        rows4[:, i * S:(i + 1) * S] for i in range(4))
                    for qh in range(2):
                        sl = slice(qh * 512, qh * 512 + 512)
                        nc.vector.tensor_scalar_mul(mrow[:, sl], st[qh][:, :],
                                                    1.0 / n_feat)
                        nc.vector.tensor_scalar_mul(vrow[:, sl], st[2 + qh][:, :],
                                                    1.0 / n_feat)
                    # var = E[x^2] - mean^2 + eps ; rstd = 1/sqrt(var)
                    nc.vector.tensor_tensor(out=srow[:, :], in0=mrow[:, :],
                                            in1=mrow[:, :], op=OP.mult)
                    nc.vector.tensor_tensor(out=vrow[:, :], in0=vrow[:, :],
                                            in1=srow[:, :], op=OP.subtract)
                    nc.vector.tensor_scalar_add(vrow[:, :], vrow[:, :], EPS)
                    nc.scalar.activation(srow[:, :], vrow[:, :], AF.Sqrt)
                    nc.vector.reciprocal(rrow[:, :], srow[:, :])
                    mb = bcp.tile([128, S], F32, tag="bc", name=f"mb_{kind}")
                    rb_ = bcp.tile([128, S], F32, tag="bc", name=f"rb_{kind}")
                    nc.gpsimd.partition_broadcast(mb[:, :], mrow[:, :])
                    nc.gpsimd.partition_broadcast(rb_[:, :], rrow[:, :])
                    for mi in range(nmt):
                        dsl = destT[:, mi * S:(mi + 1) * S]
                        nc.vector.tensor_tensor(out=dsl, in0=dsl, in1=mb[:, :],
                                                op=OP.subtract)
                        nc.vector.tensor_tensor(out=dsl, in0=dsl, in1=rb_[:, :],
                                                op=OP.mult)

            # ======== phase 2: per-group projections + attention ========
            with tc.tile_pool(name="otp", bufs=1) as otp:
              oT = otp.tile([128, HPC * S], BF16)      # normalized o^T (2048, 1024)
              with tc.tile_pool(name="gq2", bufs=3) as gqp, \
                 tc.tile_pool(name="gkn", bufs=2) as gknp, \
                 tc.tile_pool(name="gv", bufs=2) as gvp, \
                 tc.tile_pool(name="wq", bufs=3) as wqp, \
                 tc.tile_pool(name="wk", bufs=3) as wkp, \
                 tc.tile_pool(name="wv", bufs=3) as wvp, \
                 tc.tile_pool(name="rshp", bufs=2) as rshp, \
                 tc.tile_pool(name="pp", bufs=4) as ppool, \
                 tc.tile_pool(name="rsp", bufs=2) as rsp, \
                 tc.tile_pool(name="rbp", bufs=2) as rbp, \
                 tc.tile_pool(name="pjps", bufs=2, space="PSUM") as pjps, \
                 tc.tile_pool(name="sps", bufs=2, space="PSUM") as sps, \
                 tc.tile_pool(name="ops", bufs=2, space="PSUM") as ops, \
                 tc.tile_pool(name="smps", bufs=2, space="PSUM") as smps:
                for g in range(NG):
                    # ---- q^T for this group: 3 m-tiles (2x nope, 1x pe pair) ----
                    qT = gqp.tile([128, 3 * S], BF16, tag="qT")
                    for m in range(3):
                        wt = wqp.tile([128, 12, 128], BF16, tag="wqb",
                                      name=f"wqb_{g}_{m}")
                        nc.sync.dma_start(
                            out=wt[:, :, :],
                            in_=wqb_r[g, m].rearrange("p (k c) -> p k c", k=12))
                        for qh in range(2):
                            ps = pjps.tile([128, 512], F32, tag="pj")
                            for k in range(12):
                                nc.tensor.matmul(
                                    ps[:, :], wt[:, k, :],
                                    q_anT[:, k * S + qh * 512: k * S + qh * 512 + 512],
                                    start=(k == 0), stop=(k == 11))
                            nc.scalar.copy(
                                qT[:, m * S + qh * 512: m * S + qh * 512 + 512],
                                ps[:, :])
                    # rope the pe tile (m=2): rows 0:64 = head0 pe, 64:128 = head1 pe
                    pe = qT[:, 2 * S:3 * S]
                    rsh = rshp.tile([128, S], BF16, tag="rsh")
                    nc.vector.stream_shuffle(rsh[:, :], pe, SHUF_MASK)
                    nc.vector.tensor_tensor(out=rsh[:, :], in0=rsh[:, :],
                                            in1=ssq_t[:, :], op=OP.mult)
                    nc.vector.tensor_tensor(out=pe, in0=pe, in1=csq_t[:, :],
                                            op=OP.mult)
                    nc.vector.tensor_tensor(out=pe, in0=pe, in1=rsh[:, :],
                                            op=OP.add)

                    # ---- k_nope^T: 2 m-tiles ----
                    knT = gknp.tile([128, 2 * S], BF16, tag="knT")
                    for m in range(2):
                        wt = wkp.tile([128, 4, 128], BF16, tag="wk",
                                      name=f"wk_{g}_{m}")
                        nc.sync.dma_start(
                            out=wt[:, :, :],
                            in_=wkvbk_r[g, m].rearrange("p (k c) -> p k c", k=4))
                        for qh in range(2):
                            ps = pjps.tile([128, 512], F32, tag="pj")
                            for k in range(4):
                                nc.tensor.matmul(
                                    ps[:, :], wt[:, k, :],
                                    kv_cnT[:, k * S + qh * 512: k * S + qh * 512 + 512],
                                    start=(k == 0), stop=(k == 3))
                            nc.scalar.copy(
                                knT[:, m * S + qh * 512: m * S + qh * 512 + 512],
                                ps[:, :])

                    # ---- v token-major: (128 tok, 8 toktile x 256 cols) ----
                    v_sb = gvp.tile([128, TOKT * G * VD], BF16, tag="v")
                    wv_t = wvp.tile([128, 4, 256], BF16, tag="wv", name=f"wv_{g}")
                    nc.sync.dma_start(
                        out=wv_t[:, :, :],
                        in_=wkvbv_r[g].rearrange("p (k c) -> p k c", k=4))
                    for t in range(TOKT):
                        ps = pjps.tile([128, 512], F32, tag="pj")
                        for k in range(4):
                            nc.tensor.matmul(
                                ps[:, :256],
                                kv_cnT[:, k * S + t * 128: k * S + (t + 1) * 128],
                                wv_t[:, k, :], start=(k == 0), stop=(k == 3))
                        nc.scalar.copy(v_sb[:, t * 256:(t + 1) * 256],
                                       ps[:, :256])

                    # ---- attention per head ----
                    for hh in range(G):
                        hg = g * G + hh
                        po = [ops.tile([128, 512], F32, tag="po",
                                       name=f"po_{hg}_{qh}") for qh in range(2)]
                        psm = [smps.tile([1, 512], F32, tag="psm",
                                         name=f"psm_{hg}_{qh}") for qh in range(2)]
                        for qh in range(2):
                            last_ik = 4 * qh + 3
                            for ik in range(last_ik + 1):
                                qstart = 128 * ik
                                lo = max(qstart, 512 * qh)
                                hi = 512 * (qh + 1)
                                w = hi - lo
                                ps_s = sps.tile([128, 512], F32, tag="ps")
                                nc.tensor.matmul(
                                    ps_s[:, :w],
                                    knT[:, hh * S + ik * 128: hh * S + (ik + 1) * 128],
                                    qT[:, hh * S + lo: hh * S + hi],
                                    start=True, stop=False)
                                nc.tensor.matmul(
                                    ps_s[:, :w],
                                    kpeT2[hh * 64:(hh + 1) * 64, ik * 128:(ik + 1) * 128],
                                    qT[hh * 64:(hh + 1) * 64, 2 * S + lo: 2 * S + hi],
                                    start=False, stop=True)
                                p = ppool.tile([128, 512], BF16, tag="p")
                                nc.scalar.activation(p[:, :w], ps_s[:, :w],
                                                     AF.Exp, scale=SCALE)
                                if lo == qstart:
                                    nc.vector.tensor_tensor(
                                        out=p[:, 0:128], in0=p[:, 0:128],
                                        in1=tri_t[:, :], op=OP.mult)
                                nc.tensor.matmul(
                                    psm[qh][:, lo - 512 * qh: hi - 512 * qh],
                                    ones_t[:, :], p[:, :w],
                                    start=(ik == 0), stop=(ik == last_ik))
                                nc.tensor.matmul(
                                    po[qh][:, lo - 512 * qh: hi - 512 * qh],
                                    v_sb[:, ik * 256 + hh * 128: ik * 256 + (hh + 1) * 128],
                                    p[:, :w],
                                    start=(ik == 0), stop=(ik == last_ik))
                        rs = rsp.tile([1, S], F32, tag="rs")
                        nc.vector.reciprocal(rs[:, 0:512], psm[0][:, :])
                        nc.vector.reciprocal(rs[:, 512:1024], psm[1][:, :])
                        rb = rbp.tile([128, S], F32, tag="rb")
                        nc.gpsimd.partition_broadcast(rb[:, :], rs[:, :])
                        for qh in range(2):
                            nc.vector.tensor_tensor(
                                out=oT[:, hg * S + qh * 512: hg * S + qh * 512 + 512],
                                in0=po[qh][:, :],
                                in1=rb[:, qh * 512: qh * 512 + 512], op=OP.mult)

            # ======== phase 3: out^T = Wo^T @ o ========
            with tc.tile_pool(name="wop", bufs=3) as wop, \
                 tc.tile_pool(name="op", bufs=3) as outp, \
                 tc.tile_pool(name="wops", bufs=2, space="PSUM") as wops:
                for hr in range(H // 128):
                    wt = wop.tile([128, 16, 128], BF16, tag="wo", name=f"wo_{hr}")
                    nc.sync.dma_start(
                        out=wt[:, :, :],
                        in_=wo_r[hr].rearrange("p (m c) -> p m c", m=16))
                    for qh in range(2):
                        ps = wops.tile([128, 512], F32, tag="pw")
                        for m in range(HPC * VD // 128):
                            nc.tensor.matmul(
                                ps[:, :], wt[:, m, :],
                                oT[:, m * S + qh * 512: m * S + qh * 512 + 512],
                                start=(m == 0), stop=(m == HPC * VD // 128 - 1))
                        ot = outp.tile([128, 512], F32, tag="out")
                        nc.scalar.copy(ot[:, :], ps[:, :])
                        nc.sync.dma_start(
                            out=outT[hr * 128:(hr + 1) * 128, qh * 512:(qh + 1) * 512],
                            in_=ot[:, :])
    nc.compile()
    return nc


def _to_pkc(w, n_k):
    """(n_k*128, C) -> contiguous (128, n_k*C) bf16 ([p, k*c] layout)."""
    kk, c = w.shape[0] // 128, w.shape[1]
    assert kk == n_k
    return np.ascontiguousarray(
        w.reshape(n_k, 128, c).transpose(1, 0, 2).reshape(128, n_k * c)
    ).astype(NPBF)


def _host_prep(inputs):
    hs = np.asarray(inputs["hidden_states"], np.float32)
    cos = np.asarray(inputs["cos"], np.float32)
    sin = np.asarray(inputs["sin"], np.float32)
    pid = np.asarray(inputs["position_ids"]).astype(np.int64)
    Wqa = np.asarray(inputs["Wqa"], np.float32)
    gqa = np.asarray(inputs["gqa"], np.float32)
    Wqb = np.asarray(inputs["Wqb"], np.float32)
    Wkva = np.asarray(inputs["Wkva"], np.float32)
    gkva = np.asarray(inputs["gkva"], np.float32)
    Wkvb = np.asarray(inputs["Wkvb"], np.float32)
    Wo = np.asarray(inputs["Wo"], np.float32)

    # Wa = [Wqa | Wkva(kv) | Wkva(pe, rope-permuted) | LN-mean cols | pad]
    # cols 2112/2113 (locals 64/65 of the pe m-tile) carry wbar_qa/QL and
    # wbar_kv/KVL so the pe projection also produces both LN means.
    pad = np.zeros((H, 64), np.float32)
    pad[:, 0] = Wqa.mean(axis=1)
    pad[:, 1] = Wkva[:, :KVL].mean(axis=1)
    wa = np.concatenate(
        [Wqa, Wkva[:, :KVL], Wkva[:, KVL:][:, DIMS_PERM], pad], axis=1)
    # fold LN gains into the B-projections (bias terms are zero per spec)
    Wqb = Wqb * gqa[:, None]
    Wkvb = Wkvb * gkva[:, None]

    # wa_r: (17, 128, 32*128) bf16, [m][p, k*c], ordered as M_TILES (pe first)
    wa_bf = wa.astype(NPBF)
    wa_r = np.ascontiguousarray(
        wa_bf.reshape(KH, 128, MT, 128).transpose(2, 1, 0, 3)
        .reshape(MT, 128, KH * 128)[[16] + list(range(16))])

    # sign pattern for the shuffle-based rotate_half
    sign = np.where(DIMS_PERM < RD // 2, -1.0, 1.0).astype(np.float32)[:, None]

    tri = np.zeros((128, 128), np.float32)
    kp, q = np.mgrid[0:128, 0:128]
    tri[q >= kp] = 1.0
    tri = tri.astype(NPBF)

    w4 = Wqb.reshape(QL, NH, QHD)
    wk4 = Wkvb.reshape(KVL, NH, ND + VD)

    tp_data = []
    for t in range(TP):
        heads = slice(t * HPC, (t + 1) * HPC)
        # Wqb: group-blocked [h0 nope | h1 nope | h0 pe' | h1 pe'] per group
        wq = w4[:, heads]                       # (QL, 16, 192)
        nope = wq[:, :, :ND]                    # (QL, 16, 128)
        pe = wq[:, :, ND:][:, :, DIMS_PERM]     # (QL, 16, 64) permuted
        blocks = []
        for g in range(NG):
            blocks.extend([nope[:, 2 * g], nope[:, 2 * g + 1],
                           pe[:, 2 * g], pe[:, 2 * g + 1]])
        wqb_c = np.concatenate(blocks, axis=1)  # (QL, 16*192=3072)
        # (12k,128p, 8g, 3m, 128c) -> (g, m, p, k*c)
        wqb_r = np.ascontiguousarray(
            wqb_c.astype(NPBF).reshape(12, 128, NG, 3, 128)
            .transpose(2, 3, 1, 0, 4).reshape(NG, 3, 128, 12 * 128))

        wkc = wk4[:, heads]
        wkvbk_c = wkc[:, :, :ND].reshape(KVL, HPC * ND)
        wkvbv_c = wkc[:, :, ND:].reshape(KVL, HPC * VD)
        wkvbk_r = np.ascontiguousarray(
            wkvbk_c.astype(NPBF).reshape(4, 128, NG, 2, 128)
            .transpose(2, 3, 1, 0, 4).reshape(NG, 2, 128, 4 * 128))
        wkvbv_r = np.ascontiguousarray(
            wkvbv_c.astype(NPBF).reshape(4, 128, NG, 256)
            .transpose(2, 1, 0, 3).reshape(NG, 128, 4 * 256))

        wo_c = Wo[t * HPC * VD:(t + 1) * HPC * VD]    # (2048, 4096)
        wo_r = np.ascontiguousarray(
            wo_c.astype(NPBF).reshape(16, 128, KH, 128)
            .transpose(2, 1, 0, 3).reshape(KH, 128, 16 * 128))
        tp_data.append((wqb_r, wkvbk_r, wkvbv_r, wo_r))

    per_core = []
    for c in range(NCORES):
        b, t = divmod(c, TP)
        wqb_r, wkvbk_r, wkvbv_r, wo_r = tp_data[t]

        cos_g = cos[pid[b]]                     # (S, RD)
        sin_g = sin[pid[b]]
        cosT = np.ascontiguousarray(cos_g.T[DIMS_PERM])   # (64, S)
        sinT = np.ascontiguousarray(sin_g.T[DIMS_PERM])
        csq_c = np.ascontiguousarray(np.vstack([cosT, cosT])).astype(NPBF)
        ssq_c = np.ascontiguousarray(np.vstack([sinT * sign, sinT * sign])).astype(NPBF)

        hsT = hs[b].T                            # (H, S)
        hs_rc = np.ascontiguousarray(
            hsT.astype(NPBF).reshape(KH, 128, S).transpose(1, 0, 2)
            .reshape(128, KH * S))

        per_core.append({
            "hs_r": hs_rc,
            "wa_r": wa_r,
            "wqb_r": wqb_r,
            "wkvbk_r": wkvbk_r,
            "wkvbv_r": wkvbv_r,
            "wo_r": wo_r,
            "csq": csq_c,
            "ssq": ssq_c,
            "tri": tri,
            "ones_in": np.ones((128, 1), NPBF),
        })
    return per_core


def kernel(**inputs):
    if "nc" not in _NC_CACHE:
        _NC_CACHE["nc"] = _build_nc()
    nc = _NC_CACHE["nc"]
    in_maps = _host_prep(inputs)
    res = bass_utils.run_bass_kernel_spmd(nc, in_maps, core_ids=list(range(NCORES)))
    outs = []
    for b in range(B):
        acc = res.results[TP * b]["outT"].astype(np.float32)
        for t in range(1, TP):
            acc = acc + res.results[TP * b + t]["outT"]
        outs.append(acc.T)
    return np.stack(outs, axis=0)
